# revision 23
# baseline (speedup 1.0000x reference)
"""MultiHeadAttention (B=2, S=2048, HID=1024, 16 heads) on 8 TRN2 NeuronCores.

Sharding: 2 batches x 4 head-groups (4 heads each). Each core:
  - projects Q chunk 0 first, then streams K,V chunks (full-HID contraction;
    d-major kT, s-major v with fused ones columns for the softmax sum) while
    the first attention unit (head-pair 0, sq-chunk 0) pipelines against the
    arriving K/V tiles,
  - per sq-chunk: scoresT = kT q (head pairs packed into adjacent PSUM banks
    via PE row tiling), exp on ACT (scale=1/8 folded in; no max-subtraction,
    scores are ~N(0,1)), ctxT + sumexp via one v|ones matmul (M=65),
    normalize with DVE reciprocal + GPSIMD partition-broadcast,
  - partial output projection ctx_g @ Wo[rows_g], software-pipelined one
    sq-chunk behind attention.
Host sums the 4 partials per batch and adds bo (pure unshard).

All matmuls run in float32r (TRN2 single-pass reduced-mantissa fp32, RNE to
11 explicit mantissa bits); operands are pre-rounded on host or produced by
on-device ops with float32r output dtype (the BIR verifier requires rounded
producers).
"""
import numpy as np

import concourse.bacc as bacc
import concourse.mybir as mybir
import concourse.tile as tile
from concourse.bass_utils import run_bass_kernel_spmd

B, S, HID = 2, 2048, 1024
NHEAD_TOT, D = 16, 64
NG = 4                 # head-groups (tensor-parallel)
DG = HID // NG         # 256 d-columns per group
NHP = NHEAD_TOT // NG  # 4 heads per core
NPAIR = NHP // 2       # 2 head pairs per core
CS, NCH = 256, S // 256   # projection s-chunking
QS, NQ = 512, S // 512    # attention sq-chunking
NT = S // 128             # sk tiles
KT = HID // 128           # hid k-tiles

F32 = mybir.dt.float32
F32R = mybir.dt.float32r
AF = mybir.ActivationFunctionType

_NC_CACHE = {}


def round_fp32r(x: np.ndarray) -> np.ndarray:
    """RNE to 11 explicit mantissa bits (matches walrus cast_fp32_to_fp32r)."""
    x = np.ascontiguousarray(x, dtype=np.float32)
    b = x.view(np.uint32).astype(np.uint64)
    lsb = (b >> 12) & 1
    r = (b + 2047 + lsb) & ~np.uint64(0xFFF)
    return r.astype(np.uint32).view(np.float32).reshape(x.shape)


def _build():
    nc = bacc.Bacc("TRN2", target_bir_lowering=False, debug=False, num_devices=8)

    xq_d = nc.dram_tensor("xq", [NCH, 128, KT, CS], F32R, kind="ExternalInput")
    xk_d = nc.dram_tensor("xk", [NCH, 128, KT, CS], F32R, kind="ExternalInput")
    xv_d = nc.dram_tensor("xv", [NCH, 128, KT, CS], F32R, kind="ExternalInput")
    wq_d = nc.dram_tensor("wq", [128, KT, DG], F32R, kind="ExternalInput")
    wk_d = nc.dram_tensor("wk", [128, KT, DG], F32R, kind="ExternalInput")
    wv_d = nc.dram_tensor("wv", [128, KT, DG], F32R, kind="ExternalInput")
    wo_d = nc.dram_tensor("wo", [128, NPAIR, HID], F32R, kind="ExternalInput")
    bq_d = nc.dram_tensor("bq", [128, 2], F32, kind="ExternalInput")
    bk_d = nc.dram_tensor("bk", [128, 2], F32, kind="ExternalInput")
    bvbc_d = nc.dram_tensor("bvbc", [128, DG], F32, kind="ExternalInput")
    onesv_d = nc.dram_tensor("onesv", [128, NT, 2], F32R, kind="ExternalInput")
    out_d = nc.dram_tensor("out", [S, HID], F32, kind="ExternalOutput")

    with nc.allow_low_precision(reason="fp32r matmul pipeline is deliberate"), \
         tile.TileContext(nc) as tc, \
         tc.tile_pool(name="persist", bufs=1) as pp:
        # weights/constants ride the SWDGE queue, parallel to the input stream
        wq = pp.tile([128, KT, DG], F32R, tag="wq")
        wk = pp.tile([128, KT, DG], F32R, tag="wk")
        wv = pp.tile([128, KT, DG], F32R, tag="wv")
        wo = pp.tile([128, NPAIR, HID], F32R, tag="wo")
        bq = pp.tile([128, 2], F32, tag="bq")
        bk = pp.tile([128, 2], F32, tag="bk")
        bvbc = pp.tile([128, DG], F32, tag="bvbc")
        for t_, d_ in [(wq, wq_d), (bq, bq_d), (wk, wk_d), (bk, bk_d),
                       (wv, wv_d), (bvbc, bvbc_d)]:
            nc.gpsimd.dma_start(t_[:], d_[:])

        qt = [pp.tile([128, S], F32R, tag=f"qt{p}", name=f"qt{p}") for p in range(NPAIR)]
        kt = [pp.tile([128, S], F32R, tag=f"kt{p}", name=f"kt{p}") for p in range(NPAIR)]
        vp = [pp.tile([128, NT, 130], F32R, tag=f"vp{p}", name=f"vp{p}") for p in range(NPAIR)]
        # ctx per pair: head 2p in partitions 0-63, head 2p+1 in 64-127
        ctp = [pp.tile([128, S], F32R, tag=f"ctp{p}", name=f"ctp{p}") for p in range(NPAIR)]
        for p in range(NPAIR):
            nc.gpsimd.dma_start(vp[p][:, :, 64:130:65], onesv_d[:])

        with (
            tc.tile_pool(name="xqs", bufs=2) as xq_pool,
            tc.tile_pool(name="et", bufs=5) as e_pool,
            tc.tile_pool(name="zz", bufs=2) as z_pool,
        ):
            xq_tiles = {}

            def load_xq(c):
                t = xq_pool.tile([128, KT, CS], F32R, tag="xq_c", name="xq_c")
                nc.sync.dma_start(t[:], xq_d[c])
                xq_tiles[c] = t

            def qproj(qc, pool, tag):
                for c in (2 * qc, 2 * qc + 1):
                    cs = slice(c * CS, (c + 1) * CS)
                    xq_c = xq_tiles.pop(c)
                    pq = pool.tile([128, 2, CS], F32, tag=tag, name="pq")
                    for i in range(2):
                        di = slice(128 * i, 128 * (i + 1))
                        for k in range(KT):
                            fl = dict(start=(k == 0), stop=(k == KT - 1))
                            nc.tensor.matmul(pq[:, i, :], wq[:, k, di],
                                             xq_c[:, k, :], **fl)
                    for i in range(2):
                        nc.vector.tensor_scalar_add(
                            qt[i][:, cs], pq[:, i, :], bq[:, i:i + 1])

            def scores_exp1(sc_pool, pr, h2, qc, t):
                qs = slice(qc * QS, (qc + 1) * QS)
                ps = sc_pool.tile([128, QS], F32, tag="ps1", name="ps1")
                hp = slice(64 * h2, 64 * (h2 + 1))
                nc.tensor.matmul(ps[:], kt[pr][hp, 128 * t:128 * (t + 1)],
                                 qt[pr][hp, qs])
                et = e_pool.tile([128, QS], F32R, tag="et1", name="et1", bufs=4)
                nc.scalar.activation(et[:], ps[:], AF.Exp, scale=0.125)
                return et

            def scores_exp(sc_pool, pr, qc, t):
                qs = slice(qc * QS, (qc + 1) * QS)
                ps = sc_pool.tile([128, 2, QS], F32, tag="ps", name="ps")
                for h2 in range(2):
                    hp = slice(64 * h2, 64 * (h2 + 1))
                    nc.tensor.matmul(ps[:, h2, :],
                                     kt[pr][hp, 128 * t:128 * (t + 1)],
                                     qt[pr][hp, qs])
                et = e_pool.tile([128, 2, QS], F32R, tag="et", name="et")
                nc.scalar.activation(et[:], ps[:], AF.Exp, scale=0.125)
                return et

            def ctx_mm(pcs, pr, t, et):
                for h2 in range(2):
                    nc.tensor.matmul(pcs[h2], vp[pr][:, t, 65 * h2:65 * h2 + 65],
                                     et[:, h2, :],
                                     start=(t == 0), stop=(t == NT - 1))

            def normalize(pcs, pr, qc):
                qs = slice(qc * QS, (qc + 1) * QS)
                for h2 in range(2):
                    pc = pcs[h2]
                    zr = z_pool.tile([1, QS], F32R, tag="zr", name="zr")
                    nc.vector.reciprocal(zr[:], pc[64:65, :])
                    bcs = z_pool.tile([64, QS], F32R, tag="bcs", name="bcs")
                    nc.gpsimd.partition_broadcast(bcs[:], zr[:])
                    if h2 == 0:
                        nc.vector.tensor_mul(ctp[pr][0:64, qs], pc[0:64, :], bcs[:])
                    else:
                        tmp = z_pool.tile([64, QS], F32R, tag="tmp", name="tmp")
                        nc.vector.tensor_mul(tmp[:], pc[0:64, :], bcs[:])
                        nc.sync.dma_start(ctp[pr][64:128, qs], tmp[:])

            def new_pcs(pool):
                return [pool.tile([65, QS], F32, tag="pc", name=f"pc{h2}")
                        for h2 in range(2)]

            # ---- streaming phase: Q chunk 0, then K/V chunks with the first
            # attention unit (pr=0, qc=0) pipelined against arriving tiles ----
            load_xq(0)
            load_xq(1)
            with (
                tc.tile_pool(name="xs", bufs=2) as xs_pool,
                tc.tile_pool(name="pj", bufs=1, space="PSUM") as pj,
                tc.tile_pool(name="scA", bufs=3, space="PSUM") as scA,
                tc.tile_pool(name="pcA", bufs=4, space="PSUM") as ctxA_ps,
            ):
                qproj(0, pj, "pkq")
                pcs0 = [new_pcs(ctxA_ps) for _ in range(NPAIR)]
                for c in range(NCH):
                    if c == 6:
                        load_xq(2)
                        load_xq(3)
                    cs = slice(c * CS, (c + 1) * CS)
                    xk_c = xs_pool.tile([128, KT, CS], F32R, tag="xk_c", name="xk_c")
                    xv_c = xs_pool.tile([128, KT, CS], F32R, tag="xv_c", name="xv_c")
                    nc.sync.dma_start(xk_c[:], xk_d[c])
                    nc.sync.dma_start(xv_c[:], xv_d[c])

                    pk = pj.tile([128, 2, CS], F32, tag="pkq", name="pk", bufs=1)
                    for i in range(2):
                        di = slice(128 * i, 128 * (i + 1))
                        for k in range(KT):
                            fl = dict(start=(k == 0), stop=(k == KT - 1))
                            nc.tensor.matmul(pk[:, i, :], wk[:, k, di], xk_c[:, k, :], **fl)
                    for i in range(2):
                        nc.vector.tensor_scalar_add(kt[i][:, cs], pk[:, i, :], bk[:, i:i + 1])
                    pv = pj.tile([128, 2, DG], F32, tag="pkq", name="pv", bufs=1)
                    for i in range(2):
                        di = slice(128 * i, 128 * (i + 1))
                        for k in range(KT):
                            fl = dict(start=(k == 0), stop=(k == KT - 1))
                            nc.tensor.matmul(pv[:, i, :], xv_c[:, k, di], wv[:, k, :], **fl)
                    for j in range(2):          # s-slice within chunk
                        tj = 2 * c + j
                        for p in range(NPAIR):  # head pair
                            dst = vp[p][:, tj, 0:130]
                            dst = dst.rearrange("q (two x) -> q two x", two=2)[:, :, 0:64]
                            src = pv[:, j, 128 * p:128 * (p + 1)]
                            src = src.rearrange("q (two x) -> q two x", two=2)
                            bsrc = bvbc[:, 128 * p:128 * (p + 1)]
                            bsrc = bsrc.rearrange("q (two x) -> q two x", two=2)
                            nc.vector.scalar_tensor_tensor(
                                dst, src, 1.0, bsrc,
                                op0=mybir.AluOpType.mult, op1=mybir.AluOpType.add)
                    # qc=0 attention (both pairs) chases the K/V stream
                    for t in (2 * c, 2 * c + 1):
                        for pr in range(NPAIR):
                            for h2 in range(2):
                                et = scores_exp1(scA, pr, h2, 0, t)
                                nc.tensor.matmul(
                                    pcs0[pr][h2],
                                    vp[pr][:, t, 65 * h2:65 * h2 + 65], et[:],
                                    start=(t == 0), stop=(t == NT - 1))
                qproj(1, pj, "pkq")
                for pr in range(NPAIR):
                    normalize(pcs0[pr], pr, 0)

            # ---- main pipeline over remaining units ----
            nc.gpsimd.dma_start(wo[:], wo_d[:])
            with (
                tc.tile_pool(name="ob", bufs=2) as o_pool,
                tc.tile_pool(name="pqB", bufs=1, space="PSUM") as pq_ps,
                tc.tile_pool(name="scB", bufs=2, space="PSUM") as scB,
                tc.tile_pool(name="pcB", bufs=2, space="PSUM") as ctxB_ps,
                tc.tile_pool(name="po", bufs=1, space="PSUM") as out_ps,
            ):
                def attn(pr, qc):
                    ets = [scores_exp(scB, pr, qc, t) for t in range(NT)]
                    pcs = new_pcs(ctxB_ps)
                    for t in range(NT):
                        ctx_mm(pcs, pr, t, ets[t])
                    normalize(pcs, pr, qc)

                def outproj(qc):
                    for m in range(4 * qc, 4 * qc + 4):
                        ms = slice(128 * m, 128 * (m + 1))
                        ob = o_pool.tile([128, HID], F32, tag="ob", name="ob")
                        for n in range(2):
                            ns = slice(512 * n, 512 * (n + 1))
                            po = out_ps.tile([128, 512], F32, tag="po", name="po")
                            for p in range(NPAIR):
                                nc.tensor.matmul(po[:], ctp[p][:, ms], wo[:, p, ns],
                                                 start=(p == 0), stop=(p == NPAIR - 1))
                            nc.vector.tensor_copy(ob[:, ns], po[:])
                        nc.sync.dma_start(out_d[ms, :], ob[:])

                for qc in range(1, NQ):
                    attn(0, qc)
                    if qc + 1 < NQ:
                        load_xq(2 * qc + 2)
                        load_xq(2 * qc + 3)
                        qproj(qc + 1, pq_ps, "pq")
                        attn(1, qc)
                        outproj(qc - 1)
                    else:
                        outproj(qc - 1)
                        attn(1, qc)
                outproj(NQ - 1)

    nc.compile()
    return nc


def _get_nc():
    if "nc" not in _NC_CACHE:
        _NC_CACHE["nc"] = _build()
    return _NC_CACHE["nc"]


def _prep_x(x_b: np.ndarray) -> np.ndarray:
    """[S, HID] -> fp32r-rounded [NCH, 128, KT, CS] tiling of x_b^T."""
    xt = np.ascontiguousarray(x_b.T)                    # [HID, S]
    arr = xt.reshape(KT, 128, NCH, CS).transpose(2, 1, 0, 3)
    return round_fp32r(np.ascontiguousarray(arr))


def kernel(Q, K, V, Wq, bq, Wk, bk, Wv, bv, Wo, bo):
    Q, K, V = (np.asarray(a, np.float32) for a in (Q, K, V))
    Wq, Wk, Wv, Wo = (np.asarray(a, np.float32) for a in (Wq, Wk, Wv, Wo))
    bq, bk, bv, bo = (np.asarray(a, np.float32) for a in (bq, bk, bv, bo))

    nc = _get_nc()

    xqs = [_prep_x(Q[b]) for b in range(B)]
    xks = [_prep_x(K[b]) for b in range(B)]
    xvs = [_prep_x(V[b]) for b in range(B)]

    onesv = np.ones((128, NT, 2), np.float32)

    in_maps = []
    for c in range(8):
        b, g = c // NG, c % NG
        gs = slice(DG * g, DG * (g + 1))
        wq_g = round_fp32r(Wq[:, gs].reshape(KT, 128, DG).transpose(1, 0, 2))
        wk_g = round_fp32r(Wk[:, gs].reshape(KT, 128, DG).transpose(1, 0, 2))
        wv_g = round_fp32r(Wv[:, gs].reshape(KT, 128, DG).transpose(1, 0, 2))
        # wo rows grouped per head-pair: [128 (d within pair), NPAIR, HID]
        wo_g = round_fp32r(Wo[gs, :].reshape(NPAIR, 128, HID).transpose(1, 0, 2))
        in_maps.append({
            "xq": xqs[b], "xk": xks[b], "xv": xvs[b],
            "wq": wq_g, "wk": wk_g, "wv": wv_g, "wo": wo_g,
            "bq": np.ascontiguousarray(bq[gs].reshape(2, 128).T),
            "bk": np.ascontiguousarray(bk[gs].reshape(2, 128).T),
            "bvbc": np.ascontiguousarray(np.broadcast_to(bv[gs], (128, DG))),
            "onesv": onesv,
        })

    res = run_bass_kernel_spmd(nc, in_maps, core_ids=list(range(8)))

    out = np.zeros((B, S, HID), np.float32)
    for c in range(8):
        out[c // NG] += res.results[c]["out"]
    out += bo
    return out


# revision 32
# speedup vs baseline: 1.0204x; 1.0204x over previous
"""MultiHeadAttention (B=2, S=2048, HID=1024, 16 heads) on 8 TRN2 NeuronCores.

Sharding: 2 batches x 4 head-groups (4 heads each). Each core:
  - projects Q chunk 0 first, then streams K,V chunks (full-HID contraction;
    d-major kT, s-major v with fused ones columns for the softmax sum) while
    the first attention unit (head-pair 0, sq-chunk 0) pipelines against the
    arriving K/V tiles,
  - per sq-chunk: scoresT = kT q (head pairs packed into adjacent PSUM banks
    via PE row tiling), exp on ACT (scale=1/8 folded in; no max-subtraction,
    scores are ~N(0,1)), ctxT + sumexp via one v|ones matmul (M=65),
    normalize with DVE reciprocal + GPSIMD partition-broadcast,
  - partial output projection ctx_g @ Wo[rows_g], software-pipelined one
    sq-chunk behind attention.
Host sums the 4 partials per batch and adds bo (pure unshard).

All matmuls run in float32r (TRN2 single-pass reduced-mantissa fp32, RNE to
11 explicit mantissa bits); operands are pre-rounded on host or produced by
on-device ops with float32r output dtype (the BIR verifier requires rounded
producers).
"""
import numpy as np

import concourse.bacc as bacc
import concourse.mybir as mybir
import concourse.tile as tile
from concourse.bass_utils import run_bass_kernel_spmd

B, S, HID = 2, 2048, 1024
NHEAD_TOT, D = 16, 64
NG = 4                 # head-groups (tensor-parallel)
DG = HID // NG         # 256 d-columns per group
NHP = NHEAD_TOT // NG  # 4 heads per core
NPAIR = NHP // 2       # 2 head pairs per core
CS, NCH = 256, S // 256   # projection s-chunking
QS, NQ = 512, S // 512    # attention sq-chunking
NT = S // 128             # sk tiles
KT = HID // 128           # hid k-tiles

F32 = mybir.dt.float32
F32R = mybir.dt.float32r
AF = mybir.ActivationFunctionType

_NC_CACHE = {}


def round_fp32r(x: np.ndarray) -> np.ndarray:
    """RNE to 11 explicit mantissa bits (matches walrus cast_fp32_to_fp32r)."""
    x = np.ascontiguousarray(x, dtype=np.float32)
    b = x.view(np.uint32).astype(np.uint64)
    lsb = (b >> 12) & 1
    r = (b + 2047 + lsb) & ~np.uint64(0xFFF)
    return r.astype(np.uint32).view(np.float32).reshape(x.shape)


def _build():
    nc = bacc.Bacc("TRN2", target_bir_lowering=False, debug=False, num_devices=8)

    xq_d = nc.dram_tensor("xq", [NCH, 128, KT, CS], F32R, kind="ExternalInput")
    xk_d = nc.dram_tensor("xk", [NCH, 128, KT, CS], F32R, kind="ExternalInput")
    xv_d = nc.dram_tensor("xv", [NCH, 128, KT, CS], F32R, kind="ExternalInput")
    wq_d = nc.dram_tensor("wq", [128, KT, DG], F32R, kind="ExternalInput")
    wk_d = nc.dram_tensor("wk", [128, KT, DG], F32R, kind="ExternalInput")
    wv_d = nc.dram_tensor("wv", [128, KT, DG], F32R, kind="ExternalInput")
    wo_d = nc.dram_tensor("wo", [128, NPAIR, HID], F32R, kind="ExternalInput")
    bq_d = nc.dram_tensor("bq", [128, 2], F32, kind="ExternalInput")
    bk_d = nc.dram_tensor("bk", [128, 2], F32, kind="ExternalInput")
    bvbc_d = nc.dram_tensor("bvbc", [128, DG], F32, kind="ExternalInput")
    onesv_d = nc.dram_tensor("onesv", [128, NT, 2], F32R, kind="ExternalInput")
    out_d = nc.dram_tensor("out", [S, HID], F32, kind="ExternalOutput")

    with nc.allow_low_precision(reason="fp32r matmul pipeline is deliberate"), \
         tile.TileContext(nc) as tc, \
         tc.tile_pool(name="persist", bufs=1) as pp:
        # weights/constants ride the SWDGE queue, parallel to the input stream
        wq = pp.tile([128, KT, DG], F32R, tag="wq")
        wk = pp.tile([128, KT, DG], F32R, tag="wk")
        wv = pp.tile([128, KT, DG], F32R, tag="wv")
        wo = pp.tile([128, NPAIR, HID], F32R, tag="wo")
        bq = pp.tile([128, 2], F32, tag="bq")
        bk = pp.tile([128, 2], F32, tag="bk")
        bvbc = pp.tile([128, DG], F32, tag="bvbc")
        for t_, d_ in [(wq, wq_d), (bq, bq_d), (wk, wk_d), (bk, bk_d),
                       (wv, wv_d), (bvbc, bvbc_d)]:
            nc.gpsimd.dma_start(t_[:], d_[:])

        qt = [pp.tile([128, S], F32R, tag=f"qt{p}", name=f"qt{p}") for p in range(NPAIR)]
        kt = [pp.tile([128, S], F32R, tag=f"kt{p}", name=f"kt{p}") for p in range(NPAIR)]
        vp = [pp.tile([128, NT, 130], F32R, tag=f"vp{p}", name=f"vp{p}") for p in range(NPAIR)]
        # ctx per pair: head 2p in partitions 0-63, head 2p+1 in 64-127
        ctp = [pp.tile([128, S], F32R, tag=f"ctp{p}", name=f"ctp{p}") for p in range(NPAIR)]
        for p in range(NPAIR):
            nc.gpsimd.dma_start(vp[p][:, :, 64:130:65], onesv_d[:])

        with (
            tc.tile_pool(name="xqs", bufs=2) as xq_pool,
            tc.tile_pool(name="et", bufs=5) as e_pool,
            tc.tile_pool(name="zz", bufs=3) as z_pool,
        ):
            xq_tiles = {}

            def load_xq(c):
                t = xq_pool.tile([128, KT, CS], F32R, tag="xq_c", name="xq_c")
                nc.sync.dma_start(t[:], xq_d[c])
                xq_tiles[c] = t

            def qproj(qc, pool, tag):
                for c in (2 * qc, 2 * qc + 1):
                    cs = slice(c * CS, (c + 1) * CS)
                    xq_c = xq_tiles.pop(c)
                    pq = pool.tile([128, 2, CS], F32, tag=tag, name="pq")
                    for i in range(2):
                        di = slice(128 * i, 128 * (i + 1))
                        for k in range(KT):
                            fl = dict(start=(k == 0), stop=(k == KT - 1))
                            nc.tensor.matmul(pq[:, i, :], wq[:, k, di],
                                             xq_c[:, k, :], **fl)
                    for i in range(2):
                        nc.vector.tensor_scalar_add(
                            qt[i][:, cs], pq[:, i, :], bq[:, i:i + 1])

            def scores_exp1(sc_pool, pr, h2, qc, t):
                qs = slice(qc * QS, (qc + 1) * QS)
                ps = sc_pool.tile([128, QS], F32, tag="ps1", name="ps1")
                hp = slice(64 * h2, 64 * (h2 + 1))
                nc.tensor.matmul(ps[:], kt[pr][hp, 128 * t:128 * (t + 1)],
                                 qt[pr][hp, qs])
                et = e_pool.tile([128, QS], F32R, tag="et1", name="et1", bufs=4)
                nc.scalar.activation(et[:], ps[:], AF.Exp, scale=0.125)
                return et

            def scores_exp(sc_pool, pr, qc, t):
                qs = slice(qc * QS, (qc + 1) * QS)
                ps = sc_pool.tile([128, 2, QS], F32, tag="ps", name="ps")
                for h2 in range(2):
                    hp = slice(64 * h2, 64 * (h2 + 1))
                    nc.tensor.matmul(ps[:, h2, :],
                                     kt[pr][hp, 128 * t:128 * (t + 1)],
                                     qt[pr][hp, qs])
                et = e_pool.tile([128, 2, QS], F32R, tag="et", name="et")
                nc.scalar.activation(et[:], ps[:], AF.Exp, scale=0.125)
                return et

            def ctx_mm(pcs, pr, t, et):
                for h2 in range(2):
                    nc.tensor.matmul(pcs[h2], vp[pr][:, t, 65 * h2:65 * h2 + 65],
                                     et[:, h2, :],
                                     start=(t == 0), stop=(t == NT - 1))

            def normalize(pcs, pr, qc):
                qs = slice(qc * QS, (qc + 1) * QS)
                for h2 in range(2):
                    pc = pcs[h2]
                    zr = z_pool.tile([1, QS], F32R, tag="zr", name="zr")
                    nc.vector.reciprocal(zr[:], pc[64:65, :])
                    bcs = z_pool.tile([64, QS], F32R, tag="bcs", name="bcs")
                    nc.gpsimd.partition_broadcast(bcs[:], zr[:])
                    if h2 == 0:
                        nc.vector.tensor_mul(ctp[pr][0:64, qs], pc[0:64, :], bcs[:])
                    else:
                        tmp = z_pool.tile([64, QS], F32R, tag="tmp", name="tmp")
                        nc.vector.tensor_mul(tmp[:], pc[0:64, :], bcs[:])
                        nc.sync.dma_start(ctp[pr][64:128, qs], tmp[:])

            def new_pcs(pool):
                return [pool.tile([65, QS], F32, tag="pc", name=f"pc{h2}")
                        for h2 in range(2)]

            # ---- streaming phase: Q chunk 0, then K/V chunks with the first
            # attention unit (pr=0, qc=0) pipelined against arriving tiles ----
            load_xq(0)
            load_xq(1)
            with (
                tc.tile_pool(name="xs", bufs=2) as xs_pool,
                tc.tile_pool(name="pj", bufs=1, space="PSUM") as pj,
                tc.tile_pool(name="scA", bufs=3, space="PSUM") as scA,
                tc.tile_pool(name="pcA", bufs=4, space="PSUM") as ctxA_ps,
            ):
                qproj(0, pj, "pkq")
                pcs0 = [new_pcs(ctxA_ps) for _ in range(NPAIR)]
                for c in range(NCH):
                    if c == 6:
                        load_xq(2)
                        load_xq(3)
                    cs = slice(c * CS, (c + 1) * CS)
                    xk_c = xs_pool.tile([128, KT, CS], F32R, tag="xk_c", name="xk_c")
                    xv_c = xs_pool.tile([128, KT, CS], F32R, tag="xv_c", name="xv_c")
                    nc.sync.dma_start(xk_c[:], xk_d[c])
                    nc.sync.dma_start(xv_c[:], xv_d[c])

                    pk = pj.tile([128, 2, CS], F32, tag="pkq", name="pk", bufs=1)
                    for i in range(2):
                        di = slice(128 * i, 128 * (i + 1))
                        for k in range(KT):
                            fl = dict(start=(k == 0), stop=(k == KT - 1))
                            nc.tensor.matmul(pk[:, i, :], wk[:, k, di], xk_c[:, k, :], **fl)
                    for i in range(2):
                        nc.vector.tensor_scalar_add(kt[i][:, cs], pk[:, i, :], bk[:, i:i + 1])
                    pv = pj.tile([128, 2, DG], F32, tag="pkq", name="pv", bufs=1)
                    for i in range(2):
                        di = slice(128 * i, 128 * (i + 1))
                        for k in range(KT):
                            fl = dict(start=(k == 0), stop=(k == KT - 1))
                            nc.tensor.matmul(pv[:, i, :], xv_c[:, k, di], wv[:, k, :], **fl)
                    for j in range(2):          # s-slice within chunk
                        tj = 2 * c + j
                        for p in range(NPAIR):  # head pair
                            dst = vp[p][:, tj, 0:130]
                            dst = dst.rearrange("q (two x) -> q two x", two=2)[:, :, 0:64]
                            src = pv[:, j, 128 * p:128 * (p + 1)]
                            src = src.rearrange("q (two x) -> q two x", two=2)
                            bsrc = bvbc[:, 128 * p:128 * (p + 1)]
                            bsrc = bsrc.rearrange("q (two x) -> q two x", two=2)
                            nc.vector.scalar_tensor_tensor(
                                dst, src, 1.0, bsrc,
                                op0=mybir.AluOpType.mult, op1=mybir.AluOpType.add)
                    # qc=0 attention (both pairs) chases the K/V stream
                    for t in (2 * c, 2 * c + 1):
                        for pr in range(NPAIR):
                            for h2 in range(2):
                                et = scores_exp1(scA, pr, h2, 0, t)
                                nc.tensor.matmul(
                                    pcs0[pr][h2],
                                    vp[pr][:, t, 65 * h2:65 * h2 + 65], et[:],
                                    start=(t == 0), stop=(t == NT - 1))
                qproj(1, pj, "pkq")
                for pr in range(NPAIR):
                    normalize(pcs0[pr], pr, 0)

            # ---- main pipeline over remaining units ----
            nc.gpsimd.dma_start(wo[:], wo_d[:])
            with (
                tc.tile_pool(name="ob", bufs=3) as o_pool,
                tc.tile_pool(name="misc", bufs=2, space="PSUM") as misc_ps,
                tc.tile_pool(name="scB", bufs=2, space="PSUM") as scB,
                tc.tile_pool(name="pcB", bufs=2, space="PSUM") as ctxB_ps,
            ):
                pq_ps = misc_ps
                out_ps = misc_ps
                def attn(pr, qc):
                    ets = [scores_exp(scB, pr, qc, t) for t in range(NT)]
                    pcs = new_pcs(ctxB_ps)
                    for t in range(NT):
                        ctx_mm(pcs, pr, t, ets[t])
                    normalize(pcs, pr, qc)

                def outproj(qc):
                    for m in range(4 * qc, 4 * qc + 4):
                        ms = slice(128 * m, 128 * (m + 1))
                        ob = o_pool.tile([128, HID], F32, tag="ob", name="ob")
                        for n in range(2):
                            ns = slice(512 * n, 512 * (n + 1))
                            po = out_ps.tile([128, 512], F32, tag="misc", name="po")
                            for p in range(NPAIR):
                                nc.tensor.matmul(po[:], ctp[p][:, ms], wo[:, p, ns],
                                                 start=(p == 0), stop=(p == NPAIR - 1))
                            nc.vector.tensor_copy(ob[:, ns], po[:])
                        nc.sync.dma_start(out_d[ms, :], ob[:])

                for qc in range(1, NQ):
                    attn(0, qc)
                    if qc + 1 < NQ:
                        load_xq(2 * qc + 2)
                        load_xq(2 * qc + 3)
                        qproj(qc + 1, pq_ps, "misc")
                        attn(1, qc)
                        outproj(qc - 1)
                    else:
                        outproj(qc - 1)
                        attn(1, qc)
                outproj(NQ - 1)

    nc.compile()
    return nc


def _get_nc():
    if "nc" not in _NC_CACHE:
        _NC_CACHE["nc"] = _build()
    return _NC_CACHE["nc"]


def _prep_x(x_b: np.ndarray) -> np.ndarray:
    """[S, HID] -> fp32r-rounded [NCH, 128, KT, CS] tiling of x_b^T."""
    xt = np.ascontiguousarray(x_b.T)                    # [HID, S]
    arr = xt.reshape(KT, 128, NCH, CS).transpose(2, 1, 0, 3)
    return round_fp32r(np.ascontiguousarray(arr))


def kernel(Q, K, V, Wq, bq, Wk, bk, Wv, bv, Wo, bo):
    Q, K, V = (np.asarray(a, np.float32) for a in (Q, K, V))
    Wq, Wk, Wv, Wo = (np.asarray(a, np.float32) for a in (Wq, Wk, Wv, Wo))
    bq, bk, bv, bo = (np.asarray(a, np.float32) for a in (bq, bk, bv, bo))

    nc = _get_nc()

    xqs = [_prep_x(Q[b]) for b in range(B)]
    xks = [_prep_x(K[b]) for b in range(B)]
    xvs = [_prep_x(V[b]) for b in range(B)]

    onesv = np.ones((128, NT, 2), np.float32)

    in_maps = []
    for c in range(8):
        b, g = c // NG, c % NG
        gs = slice(DG * g, DG * (g + 1))
        wq_g = round_fp32r(Wq[:, gs].reshape(KT, 128, DG).transpose(1, 0, 2))
        wk_g = round_fp32r(Wk[:, gs].reshape(KT, 128, DG).transpose(1, 0, 2))
        wv_g = round_fp32r(Wv[:, gs].reshape(KT, 128, DG).transpose(1, 0, 2))
        # wo rows grouped per head-pair: [128 (d within pair), NPAIR, HID]
        wo_g = round_fp32r(Wo[gs, :].reshape(NPAIR, 128, HID).transpose(1, 0, 2))
        in_maps.append({
            "xq": xqs[b], "xk": xks[b], "xv": xvs[b],
            "wq": wq_g, "wk": wk_g, "wv": wv_g, "wo": wo_g,
            "bq": np.ascontiguousarray(bq[gs].reshape(2, 128).T),
            "bk": np.ascontiguousarray(bk[gs].reshape(2, 128).T),
            "bvbc": np.ascontiguousarray(np.broadcast_to(bv[gs], (128, DG))),
            "onesv": onesv,
        })

    res = run_bass_kernel_spmd(nc, in_maps, core_ids=list(range(8)))

    out = np.zeros((B, S, HID), np.float32)
    for c in range(8):
        out[c // NG] += res.results[c]["out"]
    out += bo
    return out
